# revision 2
# baseline (speedup 1.0000x reference)
"""Trainium2 Bass kernel for MessagePassingEdgeModule.

reference:
    src, dst = edge_index
    agg = concat([x[src], x[dst], edge_attr, u[batch[src]]], axis=1)  # [E, 512]
    h   = relu(agg @ W1 + b1)                                         # [E, 128]
    out = relu(h @ W2 + b2)                                           # [E, 128]

Strategy (8 cores, edge-parallel, dma_gather everywhere):
  - Nodes split in two halves (< / >= 25000) so rebased ids fit int16 — the
    index dtype dma_gather requires. Edges bucketed by (src_half, dst_half);
    each of the 4 buckets is assigned to 2 cores (capacity 2*81920 > ~160k
    expected). Host permutes edges accordingly; output is un-permuted on host.
  - Per edge, ONE 512B-row gather fetches packed [x16[src] | u16[batch[src]]]
    (per-node u is precomputed host-side), and one 256B-row gather fetches
    x16[dst]. dma_gather(transpose=True) lands rows directly as columns in
    the [feature, edge] layout both matmul layers consume — no on-chip
    transposes anywhere.
  - edge_attr is staged host-side pre-transposed ([128, EPC] f16) and
    streamed with plain DMAs; output is written feature-major [128, EPC] f16
    and un-transposed on host.
  - Layer 1: 4 accumulated matmuls per 512-edge tile (src, glob, dst, ea
    weight blocks); relu+b1 on the scalar engine into f16. Layer 2: one
    matmul (W2 stationary); relu+b2 on the scalar engine into the f16
    output tile.
"""
import sys
if '/opt/trn_rl_repo' not in sys.path:
    sys.path.insert(0, '/opt/trn_rl_repo')

from contextlib import ExitStack

import numpy as np

import concourse.bass as bass
import concourse.mybir as mybir
import concourse.tile as tile
from concourse import bacc
from concourse.bass_utils import run_bass_kernel_spmd

N_NODES = 50000
N_EDGES = 640000
N_GRAPHS = 16
D = 128
N_CORES = 8
SPLIT = 25000            # node-id split for the int16-indexed half-tables

TILE_E = 512             # edges per matmul tile (one PSUM bank)
GROUP_E = 4096           # edges per gather call
GROUP_TILES = GROUP_E // TILE_E         # 8
EPC = 81920              # edges per core
N_GROUPS = EPC // GROUP_E               # 20
E_PAD = EPC * N_CORES                   # 655360

f32 = mybir.dt.float32
f16 = mybir.dt.float16
i16 = mybir.dt.int16

_CACHE = {}


def _wrap_idx(seq):
    """[..., n] -> [..., 128, n//16] int16 in dma_gather's wrapped layout:
    unwrapped[k] = w[k % 16, k // 16], replicated to 128 partitions."""
    n = seq.shape[-1]
    lead = seq.shape[:-1]
    w = seq.reshape(*lead, n // 16, 16)
    w = np.swapaxes(w, -1, -2)                      # [..., 16, n//16]
    return np.tile(w, (*(1 for _ in lead), 8, 1)).astype(np.int16)


def _build_program(reps: int = 1, skip_gathers=False, skip_compute=False,
                   skip_stores=False):
    nc = bacc.Bacc("TRN2", target_bir_lowering=False, debug=False,
                   num_devices=N_CORES)

    # per-core gather tables (picked host-side by bucket)
    tbs_d = nc.dram_tensor("tbs", [SPLIT, 2 * D], f16, kind="ExternalInput").ap()
    tbd_d = nc.dram_tensor("tbd", [SPLIT, D], f16, kind="ExternalInput").ap()
    ea_d = nc.dram_tensor("ea", [D, EPC], f16, kind="ExternalInput").ap()
    idx_d = nc.dram_tensor("idx", [N_GROUPS, 128, GROUP_E // 16 * 2], i16,
                           kind="ExternalInput").ap()
    w1_d = nc.dram_tensor("w1", [4 * D, D], f32, kind="ExternalInput").ap()
    w2_d = nc.dram_tensor("w2", [D, D], f32, kind="ExternalInput").ap()
    b1_d = nc.dram_tensor("b1", [D], f32, kind="ExternalInput").ap()
    b2_d = nc.dram_tensor("b2", [D], f32, kind="ExternalInput").ap()
    out_d = nc.dram_tensor("out", [D, EPC], f16, kind="ExternalOutput").ap()

    IW = GROUP_E // 16   # idx cols per gather (256)

    with tile.TileContext(nc) as tc, ExitStack() as ctx:
        const = ctx.enter_context(tc.tile_pool(name="const", bufs=1))
        sb = ctx.enter_context(tc.tile_pool(name="sb", bufs=2))
        hp = ctx.enter_context(tc.tile_pool(name="hp", bufs=3))
        op = ctx.enter_context(tc.tile_pool(name="op", bufs=2))
        ps = ctx.enter_context(tc.tile_pool(name="ps", bufs=2, space="PSUM"))

        # ---- weights / biases (f32 staging -> f16 stationaries) ----
        w1_sb = const.tile([128, 4, D], f32)
        for c in range(4):
            nc.sync.dma_start(w1_sb[:, c, :], w1_d[c * D:(c + 1) * D, :])
        w2_f32 = const.tile([128, D], f32)
        nc.sync.dma_start(w2_f32[:], w2_d[:])
        w1f = const.tile([128, 4, D], f16)
        nc.vector.tensor_copy(w1f[:], w1_sb[:])
        w2f = const.tile([128, D], f16)
        nc.vector.tensor_copy(w2f[:], w2_f32[:])

        b1c = const.tile([128, 1], f32)
        nc.sync.dma_start(b1c[:], b1_d[:].rearrange("(p one) -> p one", one=1))
        b2c = const.tile([128, 1], f32)
        nc.sync.dma_start(b2c[:], b2_d[:].rearrange("(p one) -> p one", one=1))

        # ---- main loop ----
        def emit_main():
            for g in range(N_GROUPS):
                idx_sb = sb.tile([128, 2 * IW], i16, tag="idx")
                nc.sync.dma_start(idx_sb[:], idx_d[g])

                srcT = sb.tile([128, 2, GROUP_E], f16, tag="srcT")
                dstT = sb.tile([128, 1, GROUP_E], f16, tag="dstT")
                if skip_gathers:
                    nc.vector.memset(
                        srcT[:].rearrange("p a b -> p (a b)"), 0.25)
                    nc.vector.memset(
                        dstT[:].rearrange("p a b -> p (a b)"), 0.5)
                else:
                    nc.gpsimd.dma_gather(
                        srcT[:], tbs_d[:], idx_sb[:, :IW], GROUP_E, GROUP_E,
                        2 * D, transpose=True, single_packet=False)
                    nc.gpsimd.dma_gather(
                        dstT[:], tbd_d[:], idx_sb[:, IW:], GROUP_E, GROUP_E,
                        D, transpose=True, single_packet=False)

                eaT = sb.tile([128, GROUP_E], f16, tag="eaT")
                nc.sync.dma_start(eaT[:], ea_d[:, g * GROUP_E:(g + 1) * GROUP_E])

                outT = op.tile([128, GROUP_E], f16, tag="outT")
                if skip_compute:
                    nc.vector.tensor_add(outT[:, :GROUP_E], srcT[:, 0, :],
                                         dstT[:, 0, :])
                    nc.vector.tensor_add(outT[:, :GROUP_E], outT[:, :GROUP_E],
                                         eaT[:])
                else:
                    for t in range(GROUP_TILES):
                        sl = slice(t * TILE_E, (t + 1) * TILE_E)
                        h_ps = ps.tile([128, TILE_E], f32, tag="h")
                        nc.tensor.matmul(h_ps[:], w1f[:, 0, :], srcT[:, 0, sl],
                                         start=True, stop=False)
                        nc.tensor.matmul(h_ps[:], w1f[:, 3, :], srcT[:, 1, sl],
                                         start=False, stop=False)
                        nc.tensor.matmul(h_ps[:], w1f[:, 1, :], dstT[:, 0, sl],
                                         start=False, stop=False)
                        nc.tensor.matmul(h_ps[:], w1f[:, 2, :], eaT[:, sl],
                                         start=False, stop=True)

                        hT = hp.tile([128, TILE_E], f16, tag="hT")
                        nc.scalar.activation(hT[:], h_ps[:],
                                             mybir.ActivationFunctionType.Relu,
                                             bias=b1c[:])

                        o_ps = ps.tile([128, TILE_E], f32, tag="o")
                        nc.tensor.matmul(o_ps[:], w2f[:], hT[:],
                                         start=True, stop=True)
                        nc.scalar.activation(outT[:, sl], o_ps[:],
                                             mybir.ActivationFunctionType.Relu,
                                             bias=b2c[:])

                if skip_stores:
                    nc.sync.dma_start(out_d[:, g * GROUP_E:g * GROUP_E + 128],
                                      outT[:, :128])
                else:
                    nc.sync.dma_start(out_d[:, g * GROUP_E:(g + 1) * GROUP_E],
                                      outT[:])

        if reps == 1:
            emit_main()
        else:
            with tc.For_i(0, reps, 1):
                emit_main()

    nc.compile()
    return nc


def _prep_inputs(x, edge_attr, u, W1, b1, W2, b2, edge_index, batch):
    src = np.asarray(edge_index[0]).astype(np.int64)
    dst = np.asarray(edge_index[1]).astype(np.int64)
    batch = np.asarray(batch).astype(np.int64)

    x16 = np.asarray(x, np.float16)
    u16 = np.asarray(u, np.float16)
    xg = np.concatenate([x16, u16[batch]], axis=1)     # [N, 256]
    tbs0 = np.ascontiguousarray(xg[:SPLIT])
    tbs1 = np.ascontiguousarray(xg[SPLIT:])
    tbd0 = np.ascontiguousarray(x16[:SPLIT])
    tbd1 = np.ascontiguousarray(x16[SPLIT:])

    # bucket edges by (src_half, dst_half); 2 cores per bucket
    bucket = (src >= SPLIT) * 2 + (dst >= SPLIT)
    order = np.argsort(bucket, kind="stable")
    counts = np.bincount(bucket, minlength=4)

    perm = np.full(E_PAD, -1, np.int64)    # padded slot -> original edge id
    overflow = []
    pos = 0
    for b in range(4):
        ids = order[pos:pos + counts[b]]
        pos += counts[b]
        if len(ids) > 2 * EPC:
            overflow.append(ids[2 * EPC:])
            ids = ids[:2 * EPC]
        c0 = 2 * b
        n0 = min(len(ids), EPC)
        perm[c0 * EPC:c0 * EPC + n0] = ids[:n0]
        n1 = len(ids) - n0
        perm[(c0 + 1) * EPC:(c0 + 1) * EPC + n1] = ids[n0:]
    overflow = np.concatenate(overflow) if overflow else np.zeros(0, np.int64)

    valid = perm >= 0
    permv = np.where(valid, perm, 0)
    srcp = np.where(valid, src[permv] % SPLIT, 0).astype(np.int16)
    dstp = np.where(valid, dst[permv] % SPLIT, 0).astype(np.int16)

    ea16 = np.asarray(edge_attr, np.float16)
    eap = ea16[permv]
    eap[~valid] = 0

    W1f = np.asarray(W1, np.float32)
    W2f = np.asarray(W2, np.float32)
    b1f = np.asarray(b1, np.float32)
    b2f = np.asarray(b2, np.float32)

    in_maps = []
    for c in range(N_CORES):
        cs = slice(c * EPC, (c + 1) * EPC)
        bs, bd = (c // 2) >> 1, (c // 2) & 1
        iw_s = _wrap_idx(srcp[cs].reshape(N_GROUPS, GROUP_E))  # [NG,128,256]
        iw_d = _wrap_idx(dstp[cs].reshape(N_GROUPS, GROUP_E))
        idx = np.concatenate([iw_s, iw_d], axis=2)
        im = {
            "tbs": tbs1 if bs else tbs0,
            "tbd": tbd1 if bd else tbd0,
            "ea": np.ascontiguousarray(eap[cs].T),
            "idx": np.ascontiguousarray(idx),
            "w1": W1f, "w2": W2f, "b1": b1f, "b2": b2f,
        }
        in_maps.append(im)
    return in_maps, perm, overflow


def _cpu_edges(ids, x, edge_attr, u, W1, b1, W2, b2, src, dst, batch):
    agg = np.concatenate([x[src[ids]], x[dst[ids]], edge_attr[ids],
                          u[batch[src[ids]]]], axis=1).astype(np.float32)
    h = np.maximum(agg @ W1 + b1, 0)
    return np.maximum(h @ W2 + b2, 0)


def kernel(x, edge_attr, u, W1, b1, W2, b2, edge_index, batch):
    if "nc" not in _CACHE:
        _CACHE["nc"] = _build_program()
    nc = _CACHE["nc"]
    in_maps, perm, overflow = _prep_inputs(x, edge_attr, u, W1, b1, W2, b2,
                                           edge_index, batch)
    res = run_bass_kernel_spmd(nc, in_maps, list(range(N_CORES)))
    outT = np.concatenate([r["out"] for r in res.results], axis=1)  # [128,E_PAD]

    out = np.zeros((N_EDGES, D), np.float32)
    valid = perm >= 0
    out[perm[valid]] = outT.T[valid].astype(np.float32)
    if len(overflow):
        src = np.asarray(edge_index[0]).astype(np.int64)
        dst = np.asarray(edge_index[1]).astype(np.int64)
        out[overflow] = _cpu_edges(
            overflow, np.asarray(x, np.float32), np.asarray(edge_attr, np.float32),
            np.asarray(u, np.float32), np.asarray(W1, np.float32),
            np.asarray(b1, np.float32), np.asarray(W2, np.float32),
            np.asarray(b2, np.float32), src, dst,
            np.asarray(batch).astype(np.int64))
    return out


# revision 16
# speedup vs baseline: 2.0251x; 2.0251x over previous
"""Trainium2 Bass kernel for MessagePassingEdgeModule.

reference:
    src, dst = edge_index
    agg = concat([x[src], x[dst], edge_attr, u[batch[src]]], axis=1)  # [E, 512]
    h   = relu(agg @ W1 + b1)                                         # [E, 128]
    out = relu(h @ W2 + b2)                                           # [E, 128]

Strategy (8 cores, edge-parallel; HW truth: dma_gather costs ~9ns PER
DESCRIPTOR regardless of bytes/source, so descriptors are the currency):
  - Nodes split in two halves (< / >= 25000); edges bucketed by
    (src_half, dst_half), 2 cores per bucket.
  - src side needs NO gather descriptors: each core's edges are binned by
    src band (128 consecutive node ids). Tile t of group g holds only edges
    with src in band 8g+t, so the per-tile "gather" is a plain contiguous
    band load plus a one-hot expansion matmul on the PE:
        srcT_tile = xs_band^T @ S,  S[p,e] = (srcoff[e] & 127 == p)
    S is built on-chip: K=1 matmul broadcasts the host-streamed
    (srcoff&127) row across partitions, then a DVE is_equal against an
    iota column. Tiles are 448 edges (PSUM allows 512) so band capacity
    448 vs ~410 expected keeps padding ~9%.
  - dst side keeps one dma_gather per group (the unavoidable random side)
    from an SBUF-resident half-table (row n -> partition n%128, stripe
    n//128), elem 256B.
  - u[batch[src]] via one-hot matmul (Wu = u@W1u on device, host-built
    16-row one-hot stream). edge_attr pre-transposed host-side, output
    written feature-major f16 and un-permuted on host. No on-chip
    transposes anywhere; both layers run feature-major.
"""
import sys
if '/opt/trn_rl_repo' not in sys.path:
    sys.path.insert(0, '/opt/trn_rl_repo')

from contextlib import ExitStack

import numpy as np

import concourse.bass as bass
import concourse.mybir as mybir
import concourse.tile as tile
from concourse import bacc
from concourse.bass_utils import run_bass_kernel_spmd

N_NODES = 50000
N_EDGES = 640000
N_GRAPHS = 16
D = 128
N_CORES = 8
SPLIT = 25000            # node-id split for the two half-tables
TBL_RANKS = 196          # ceil(SPLIT/128) stripes for the SBUF dst table
TBL_PAD = TBL_RANKS * 128            # 25088
SRC_ROWS = 25600         # src table rows (200 bands of 128)

TILE_E = 448             # edge slots per tile (= per src band)
GROUP_TILES = 8
GROUP_E = TILE_E * GROUP_TILES       # 3584
N_GROUPS = 25            # 25 groups x 8 bands = 200 bands (196 used)
SLOTS = N_GROUPS * GROUP_E           # 89600 slots per core
BAND = 128

f32 = mybir.dt.float32
f16 = mybir.dt.float16
i16 = mybir.dt.int16

_CACHE = {}


def _wrap_idx(seq):
    """[..., n] -> [..., 128, n//16] int16 in dma_gather's wrapped layout."""
    n = seq.shape[-1]
    lead = seq.shape[:-1]
    w = seq.reshape(*lead, n // 16, 16)
    w = np.swapaxes(w, -1, -2)
    return np.tile(w, (*(1 for _ in lead), 8, 1)).astype(np.int16)


def _build_program(reps: int = 1, skip_gathers=False, skip_compute=False,
                   skip_stores=False):
    nc = bacc.Bacc("TRN2", target_bir_lowering=False, debug=False,
                   num_devices=N_CORES)

    tbs_d = nc.dram_tensor("tbs", [SRC_ROWS, D], f16, kind="ExternalInput").ap()
    tbd_d = nc.dram_tensor("tbd", [TBL_PAD, D], f16, kind="ExternalInput").ap()
    ea_d = nc.dram_tensor("ea", [D, SLOTS], f16, kind="ExternalInput").ap()
    oh_d = nc.dram_tensor("oh", [16, SLOTS], f16, kind="ExternalInput").ap()
    sm_d = nc.dram_tensor("sm", [1, SLOTS], f16, kind="ExternalInput").ap()
    idx_d = nc.dram_tensor("idx", [N_GROUPS, 128, GROUP_E // 16], i16,
                           kind="ExternalInput").ap()
    iota_d = nc.dram_tensor("iota", [128, 1], f32, kind="ExternalInput").ap()
    w1_d = nc.dram_tensor("w1", [4 * D, D], f32, kind="ExternalInput").ap()
    w2_d = nc.dram_tensor("w2", [D, D], f32, kind="ExternalInput").ap()
    b1_d = nc.dram_tensor("b1", [D], f32, kind="ExternalInput").ap()
    b2_d = nc.dram_tensor("b2", [D], f32, kind="ExternalInput").ap()
    u_d = nc.dram_tensor("u", [N_GRAPHS, D], f32, kind="ExternalInput").ap()
    out_d = nc.dram_tensor("out", [D, SLOTS], f16, kind="ExternalOutput").ap()

    IW = GROUP_E // 16   # dst idx cols per group (224)

    with tile.TileContext(nc) as tc, ExitStack() as ctx:
        const = ctx.enter_context(tc.tile_pool(name="const", bufs=1))
        sb = ctx.enter_context(tc.tile_pool(name="sb", bufs=2))
        hp = ctx.enter_context(tc.tile_pool(name="hp", bufs=3))
        op = ctx.enter_context(tc.tile_pool(name="op", bufs=2))
        ps = ctx.enter_context(tc.tile_pool(name="ps", bufs=2, space="PSUM"))

        # ---- constants ----
        from concourse.masks import make_identity
        ident = const.tile([128, 128], f32)
        make_identity(nc, ident[:])

        w1_sb = const.tile([128, 4, D], f32)
        for c in range(4):
            nc.sync.dma_start(w1_sb[:, c, :], w1_d[c * D:(c + 1) * D, :])
        w2_f32 = const.tile([128, D], f32)
        nc.sync.dma_start(w2_f32[:], w2_d[:])
        w1f = const.tile([128, 4, D], f16)
        nc.vector.tensor_copy(w1f[:], w1_sb[:])
        w2f = const.tile([128, D], f16)
        nc.vector.tensor_copy(w2f[:], w2_f32[:])

        b1c = const.tile([128, 1], f32)
        nc.sync.dma_start(b1c[:], b1_d[:].rearrange("(p one) -> p one", one=1))
        b2c = const.tile([128, 1], f32)
        nc.sync.dma_start(b2c[:], b2_d[:].rearrange("(p one) -> p one", one=1))
        iota_sb = const.tile([128, 1], f32)
        nc.sync.dma_start(iota_sb[:], iota_d[:])
        ones1 = const.tile([1, 128], f16)
        nc.vector.memset(ones1[:], 1.0)

        # Wu = u @ W1u  ([16, 128])
        u_sb = const.tile([16, D], f32)
        nc.sync.dma_start(u_sb[:], u_d[:])
        ut_ps = ps.tile([128, 16], f32, tag="h")
        nc.tensor.transpose(out=ut_ps[:], in_=u_sb[:], identity=ident[:16, :16])
        ut_sb = const.tile([128, 16], f32)
        nc.vector.tensor_copy(ut_sb[:], ut_ps[:])
        wu_ps = ps.tile([16, 128], f32, tag="o")
        nc.tensor.matmul(wu_ps[:], ut_sb[:], w1_sb[:, 3, :], start=True,
                         stop=True)
        wu_sb = const.tile([16, 128], f16)
        nc.vector.tensor_copy(wu_sb[:], wu_ps[:])

        # dst half-table resident in SBUF: row n -> partition n%128,
        # stripe n//128 (dma_gather sbuf_tokens_per_rank=128 layout)
        tbd_sb = const.tile([128, TBL_RANKS, D], f16)
        nc.sync.dma_start(tbd_sb[:],
                          tbd_d[:].rearrange("(s p) k -> p s k", p=128))

        # ---- main loop ----
        def emit_main():
            for g in range(N_GROUPS):
                idx_sb = sb.tile([128, IW], i16, tag="idx")
                nc.sync.dma_start(idx_sb[:], idx_d[g])

                xs_sb = sb.tile([128, GROUP_TILES, D], f16, tag="xs")
                nc.sync.dma_start(
                    xs_sb[:],
                    tbs_d[g * GROUP_TILES * BAND:(g + 1) * GROUP_TILES * BAND,
                          :].rearrange("(c p) k -> p c k", p=128))

                dstT = sb.tile([128, 1, GROUP_E], f16, tag="dstT")
                if skip_gathers:
                    nc.vector.memset(
                        dstT[:].rearrange("p a b -> p (a b)"), 0.5)
                else:
                    nc.gpsimd.dma_gather(
                        dstT[:], tbd_sb[:], idx_sb[:], GROUP_E, GROUP_E,
                        D, transpose=True, single_packet=False,
                        sbuf_tokens_per_rank=128,
                        sbuf_free_dim_per_rank=2 * D)

                eaT = sb.tile([128, GROUP_E], f16, tag="eaT")
                nc.sync.dma_start(eaT[:], ea_d[:, g * GROUP_E:(g + 1) * GROUP_E])
                oh_sb = sb.tile([16, GROUP_E], f16, tag="oh")
                nc.sync.dma_start(oh_sb[:], oh_d[:, g * GROUP_E:(g + 1) * GROUP_E])
                sm_sb = sb.tile([1, GROUP_E], f16, tag="sm")
                nc.sync.dma_start(sm_sb[:], sm_d[:, g * GROUP_E:(g + 1) * GROUP_E])

                outT = op.tile([128, GROUP_E], f16, tag="outT")
                if skip_compute:
                    nc.vector.tensor_add(outT[:, :GROUP_E], dstT[:, 0, :],
                                         eaT[:])
                else:
                    for t in range(GROUP_TILES):
                        sl = slice(t * TILE_E, (t + 1) * TILE_E)
                        # S[p, e] = (srcoff[e] & 127 == p)
                        bc_ps = ps.tile([128, TILE_E], f32, tag="bc")
                        nc.tensor.matmul(bc_ps[:], ones1[:], sm_sb[:, sl],
                                         start=True, stop=True)
                        S_sb = hp.tile([128, TILE_E], f16, tag="S")
                        nc.vector.tensor_scalar(
                            S_sb[:], bc_ps[:], iota_sb[:], None,
                            mybir.AluOpType.is_equal)
                        # srcT = xs_band^T @ S  (= x16[src[e]] columns)
                        x_ps = ps.tile([128, TILE_E], f32, tag="x")
                        nc.tensor.matmul(x_ps[:], xs_sb[:, t, :], S_sb[:],
                                         start=True, stop=True)
                        srcT = hp.tile([128, TILE_E], f16, tag="srcT")
                        nc.vector.tensor_copy(srcT[:], x_ps[:])

                        h_ps = ps.tile([128, TILE_E], f32, tag="h")
                        nc.tensor.matmul(h_ps[:], w1f[:, 0, :], srcT[:],
                                         start=True, stop=False)
                        nc.tensor.matmul(h_ps[:], wu_sb[:], oh_sb[:, sl],
                                         start=False, stop=False)
                        nc.tensor.matmul(h_ps[:], w1f[:, 1, :], dstT[:, 0, sl],
                                         start=False, stop=False)
                        nc.tensor.matmul(h_ps[:], w1f[:, 2, :], eaT[:, sl],
                                         start=False, stop=True)

                        hT = hp.tile([128, TILE_E], f16, tag="hT")
                        nc.scalar.activation(hT[:], h_ps[:],
                                             mybir.ActivationFunctionType.Relu,
                                             bias=b1c[:])

                        o_ps = ps.tile([128, TILE_E], f32, tag="o")
                        nc.tensor.matmul(o_ps[:], w2f[:], hT[:],
                                         start=True, stop=True)
                        nc.scalar.activation(outT[:, sl], o_ps[:],
                                             mybir.ActivationFunctionType.Relu,
                                             bias=b2c[:])

                if skip_stores:
                    nc.sync.dma_start(out_d[:, g * GROUP_E:g * GROUP_E + 128],
                                      outT[:, :128])
                else:
                    nc.sync.dma_start(out_d[:, g * GROUP_E:(g + 1) * GROUP_E],
                                      outT[:])

        if reps == 1:
            emit_main()
        else:
            with tc.For_i(0, reps, 1):
                emit_main()

    nc.compile()
    return nc


def _prep_inputs(x, edge_attr, u, W1, b1, W2, b2, edge_index, batch):
    src = np.asarray(edge_index[0]).astype(np.int64)
    dst = np.asarray(edge_index[1]).astype(np.int64)
    batch = np.asarray(batch).astype(np.int64)

    x16 = np.asarray(x, np.float16)
    tbs0 = np.zeros((SRC_ROWS, D), np.float16)
    tbs0[:SPLIT] = x16[:SPLIT]
    tbs1 = np.zeros((SRC_ROWS, D), np.float16)
    tbs1[:N_NODES - SPLIT] = x16[SPLIT:]
    tbd0 = np.zeros((TBL_PAD, D), np.float16)
    tbd0[:SPLIT] = x16[:SPLIT]
    tbd1 = np.zeros((TBL_PAD, D), np.float16)
    tbd1[:N_NODES - SPLIT] = x16[SPLIT:]

    srcoff = np.where(src >= SPLIT, src - SPLIT, src)
    dstoff = np.where(dst >= SPLIT, dst - SPLIT, dst)
    bucket = (src >= SPLIT) * 2 + (dst >= SPLIT)
    band = srcoff >> 7                          # 0..195

    key = bucket * 256 + band
    order = np.argsort(key, kind="stable")
    cnt = np.bincount(key, minlength=4 * 256).reshape(4, 256)

    perm = np.full((N_CORES, SLOTS), -1, np.int64)
    overflow = []
    pos = 0
    for b in range(4):
        for bd in range(256):
            n = cnt[b, bd]
            if n == 0:
                continue
            ids = order[pos:pos + n]
            pos += n
            n0 = min((n + 1) // 2, TILE_E)
            n1 = min(n - n0, TILE_E)
            if n0 + n1 < n:
                overflow.append(ids[n0 + n1:])
            g, t = bd // GROUP_TILES, bd % GROUP_TILES
            base = g * GROUP_E + t * TILE_E
            perm[2 * b, base:base + n0] = ids[:n0]
            perm[2 * b + 1, base:base + n1] = ids[n0:n0 + n1]
    overflow = np.concatenate(overflow) if overflow else np.zeros(0, np.int64)

    ea16 = np.asarray(edge_attr, np.float16)
    W1f = np.asarray(W1, np.float32)
    W2f = np.asarray(W2, np.float32)
    b1f = np.asarray(b1, np.float32)
    b2f = np.asarray(b2, np.float32)
    uf = np.asarray(u, np.float32)
    iota = np.arange(128, dtype=np.float32).reshape(128, 1)

    in_maps = []
    for c in range(N_CORES):
        bs, bd_half = (c // 2) >> 1, (c // 2) & 1
        pc = perm[c]
        valid = pc >= 0
        pv = np.where(valid, pc, 0)

        didx = np.where(valid, dstoff[pv], 0).astype(np.int16)
        sm = np.where(valid, srcoff[pv] & 127, 300).astype(np.float16)
        eac = ea16[pv]
        eac[~valid] = 0
        bsrc = batch[src[pv]]
        ohc = (np.arange(16)[:, None] == bsrc[None, :]).astype(np.float16)

        im = {
            "tbs": tbs1 if bs else tbs0,
            "tbd": tbd1 if bd_half else tbd0,
            "ea": np.ascontiguousarray(eac.T),
            "oh": np.ascontiguousarray(ohc),
            "sm": np.ascontiguousarray(sm.reshape(1, SLOTS)),
            "idx": np.ascontiguousarray(
                _wrap_idx(didx.reshape(N_GROUPS, GROUP_E))),
            "iota": iota,
            "w1": W1f, "w2": W2f, "b1": b1f, "b2": b2f, "u": uf,
        }
        in_maps.append(im)
    return in_maps, perm.reshape(-1), overflow


def _cpu_edges(ids, x, edge_attr, u, W1, b1, W2, b2, src, dst, batch):
    agg = np.concatenate([x[src[ids]], x[dst[ids]], edge_attr[ids],
                          u[batch[src[ids]]]], axis=1).astype(np.float32)
    h = np.maximum(agg @ W1 + b1, 0)
    return np.maximum(h @ W2 + b2, 0)


def kernel(x, edge_attr, u, W1, b1, W2, b2, edge_index, batch):
    if "nc" not in _CACHE:
        _CACHE["nc"] = _build_program()
    nc = _CACHE["nc"]
    in_maps, perm, overflow = _prep_inputs(x, edge_attr, u, W1, b1, W2, b2,
                                           edge_index, batch)
    res = run_bass_kernel_spmd(nc, in_maps, list(range(N_CORES)))
    outT = np.concatenate([r["out"] for r in res.results], axis=1)

    out = np.zeros((N_EDGES, D), np.float32)
    valid = perm >= 0
    out[perm[valid]] = outT.T[valid].astype(np.float32)
    if len(overflow):
        src = np.asarray(edge_index[0]).astype(np.int64)
        dst = np.asarray(edge_index[1]).astype(np.int64)
        out[overflow] = _cpu_edges(
            overflow, np.asarray(x, np.float32),
            np.asarray(edge_attr, np.float32),
            np.asarray(u, np.float32), np.asarray(W1, np.float32),
            np.asarray(b1, np.float32), np.asarray(W2, np.float32),
            np.asarray(b2, np.float32), src, dst,
            np.asarray(batch).astype(np.int64))
    return out


# revision 19
# speedup vs baseline: 4.3473x; 2.1467x over previous
"""Trainium2 Bass kernel for MessagePassingEdgeModule.

reference:
    src, dst = edge_index
    agg = concat([x[src], x[dst], edge_attr, u[batch[src]]], axis=1)  # [E, 512]
    h   = relu(agg @ W1 + b1)                                         # [E, 128]
    out = relu(h @ W2 + b2)                                           # [E, 128]

Strategy (8 cores, edge-parallel; HW truth: dma_gather costs ~9ns PER
DESCRIPTOR regardless of bytes/source, so descriptors are the currency):
  - Nodes split in two halves (< / >= 25000); edges bucketed by
    (src_half, dst_half), 2 cores per bucket.
  - src side needs NO gather descriptors: each core's edges are binned by
    src band (128 consecutive node ids). Tile t of group g holds only edges
    with src in band 8g+t, so the per-tile "gather" is a plain contiguous
    band load plus a one-hot expansion matmul on the PE:
        srcT_tile = xs_band^T @ S,  S[p,e] = (srcoff[e] & 127 == p)
    S is built on-chip: K=1 matmul broadcasts the host-streamed
    (srcoff&127) row across partitions, then a DVE is_equal against an
    iota column. Tiles are 448 edges (PSUM allows 512) so band capacity
    448 vs ~410 expected keeps padding ~9%.
  - dst side keeps one dma_gather per group (the unavoidable random side)
    from an SBUF-resident half-table (row n -> partition n%128, stripe
    n//128), elem 256B.
  - u[batch[src]] via one-hot matmul (Wu = u@W1u on device, host-built
    16-row one-hot stream). edge_attr pre-transposed host-side, output
    written feature-major f16 and un-permuted on host. No on-chip
    transposes anywhere; both layers run feature-major.
"""
import sys
if '/opt/trn_rl_repo' not in sys.path:
    sys.path.insert(0, '/opt/trn_rl_repo')

from contextlib import ExitStack

import numpy as np

import concourse.bass as bass
import concourse.mybir as mybir
import concourse.tile as tile
from concourse import bacc
from concourse.bass_utils import run_bass_kernel_spmd

N_NODES = 50000
N_EDGES = 640000
N_GRAPHS = 16
D = 128
N_CORES = 8
SPLIT = 25000            # node-id split for the two half-tables
TBL_RANKS = 196          # ceil(SPLIT/128) stripes for the SBUF dst table
TBL_PAD = TBL_RANKS * 128            # 25088
SRC_ROWS = 25600         # src table rows (200 bands of 128)

TILE_E = 448             # edge slots per tile (= per src band)
GROUP_TILES = 8
GROUP_E = TILE_E * GROUP_TILES       # 3584
N_GROUPS = 25            # 25 groups x 8 bands = 200 bands (196 used)
SLOTS = N_GROUPS * GROUP_E           # 89600 slots per core
BAND = 128

f32 = mybir.dt.float32
f16 = mybir.dt.float16
i16 = mybir.dt.int16

_CACHE = {}


def _wrap_idx(seq):
    """[..., n] -> [..., 128, n//16] int16 in dma_gather's wrapped layout."""
    n = seq.shape[-1]
    lead = seq.shape[:-1]
    w = seq.reshape(*lead, n // 16, 16)
    w = np.swapaxes(w, -1, -2)
    return np.tile(w, (*(1 for _ in lead), 8, 1)).astype(np.int16)


def _build_program(reps: int = 1, skip_gathers=False, skip_compute=False,
                   skip_stores=False, hbm_dst=False, sb_bufs=3, hp_bufs=4):
    nc = bacc.Bacc("TRN2", target_bir_lowering=False, debug=False,
                   num_devices=N_CORES)

    tbs_d = nc.dram_tensor("tbs", [SRC_ROWS, D], f16, kind="ExternalInput").ap()
    tbd_d = nc.dram_tensor("tbd", [TBL_PAD, D], f16, kind="ExternalInput").ap()
    ea_d = nc.dram_tensor("ea", [D, SLOTS], f16, kind="ExternalInput").ap()
    oh_d = nc.dram_tensor("oh", [16, SLOTS], f16, kind="ExternalInput").ap()
    sm_d = nc.dram_tensor("sm", [1, SLOTS], f16, kind="ExternalInput").ap()
    idx_d = nc.dram_tensor("idx", [N_GROUPS, 128, GROUP_E // 16], i16,
                           kind="ExternalInput").ap()
    iota_d = nc.dram_tensor("iota", [128, 1], f32, kind="ExternalInput").ap()
    w1_d = nc.dram_tensor("w1", [4 * D, D], f32, kind="ExternalInput").ap()
    w2_d = nc.dram_tensor("w2", [D, D], f32, kind="ExternalInput").ap()
    b1_d = nc.dram_tensor("b1", [D], f32, kind="ExternalInput").ap()
    b2_d = nc.dram_tensor("b2", [D], f32, kind="ExternalInput").ap()
    u_d = nc.dram_tensor("u", [N_GRAPHS, D], f32, kind="ExternalInput").ap()
    out_d = nc.dram_tensor("out", [D, SLOTS], f16, kind="ExternalOutput").ap()

    IW = GROUP_E // 16   # dst idx cols per group (224)

    with tile.TileContext(nc) as tc, ExitStack() as ctx:
        const = ctx.enter_context(tc.tile_pool(name="const", bufs=1))
        sb = ctx.enter_context(tc.tile_pool(name="sb", bufs=sb_bufs))
        hp = ctx.enter_context(tc.tile_pool(name="hp", bufs=hp_bufs))
        op = ctx.enter_context(tc.tile_pool(name="op", bufs=2))
        ps = ctx.enter_context(tc.tile_pool(name="ps", bufs=2, space="PSUM"))

        # ---- constants ----
        from concourse.masks import make_identity
        ident = const.tile([128, 128], f32)
        make_identity(nc, ident[:])

        w1_sb = const.tile([128, 4, D], f32)
        for c in range(4):
            nc.sync.dma_start(w1_sb[:, c, :], w1_d[c * D:(c + 1) * D, :])
        w2_f32 = const.tile([128, D], f32)
        nc.sync.dma_start(w2_f32[:], w2_d[:])
        w1f = const.tile([128, 4, D], f16)
        nc.vector.tensor_copy(w1f[:], w1_sb[:])
        w2f = const.tile([128, D], f16)
        nc.vector.tensor_copy(w2f[:], w2_f32[:])

        b1c = const.tile([128, 1], f32)
        nc.sync.dma_start(b1c[:], b1_d[:].rearrange("(p one) -> p one", one=1))
        b2c = const.tile([128, 1], f32)
        nc.sync.dma_start(b2c[:], b2_d[:].rearrange("(p one) -> p one", one=1))
        iota_sb = const.tile([128, 1], f32)
        nc.sync.dma_start(iota_sb[:], iota_d[:])
        ones1 = const.tile([1, 128], f16)
        nc.vector.memset(ones1[:], 1.0)

        # Wu = u @ W1u  ([16, 128])
        u_sb = const.tile([16, D], f32)
        nc.sync.dma_start(u_sb[:], u_d[:])
        ut_ps = ps.tile([128, 16], f32, tag="h")
        nc.tensor.transpose(out=ut_ps[:], in_=u_sb[:], identity=ident[:16, :16])
        ut_sb = const.tile([128, 16], f32)
        nc.vector.tensor_copy(ut_sb[:], ut_ps[:])
        wu_ps = ps.tile([16, 128], f32, tag="o")
        nc.tensor.matmul(wu_ps[:], ut_sb[:], w1_sb[:, 3, :], start=True,
                         stop=True)
        wu_sb = const.tile([16, 128], f16)
        nc.vector.tensor_copy(wu_sb[:], wu_ps[:])

        # dst half-table resident in SBUF: row n -> partition n%128,
        # stripe n//128 (dma_gather sbuf_tokens_per_rank=128 layout)
        if not hbm_dst:
            tbd_sb = const.tile([128, TBL_RANKS, D], f16)
            nc.sync.dma_start(tbd_sb[:],
                              tbd_d[:].rearrange("(s p) k -> p s k", p=128))

        # ---- main loop ----
        def emit_main():
            for g in range(N_GROUPS):
                idx_sb = sb.tile([128, IW], i16, tag="idx")
                nc.sync.dma_start(idx_sb[:], idx_d[g])

                xs_sb = sb.tile([128, GROUP_TILES, D], f16, tag="xs")
                nc.sync.dma_start(
                    xs_sb[:],
                    tbs_d[g * GROUP_TILES * BAND:(g + 1) * GROUP_TILES * BAND,
                          :].rearrange("(c p) k -> p c k", p=128))

                dstT = sb.tile([128, 1, GROUP_E], f16, tag="dstT")
                if skip_gathers:
                    nc.vector.memset(
                        dstT[:].rearrange("p a b -> p (a b)"), 0.5)
                elif hbm_dst:
                    nc.gpsimd.dma_gather(
                        dstT[:], tbd_d[:], idx_sb[:], GROUP_E, GROUP_E,
                        D, transpose=True, single_packet=False)
                else:
                    nc.gpsimd.dma_gather(
                        dstT[:], tbd_sb[:], idx_sb[:], GROUP_E, GROUP_E,
                        D, transpose=True, single_packet=False,
                        sbuf_tokens_per_rank=128,
                        sbuf_free_dim_per_rank=2 * D)

                eaT = sb.tile([128, GROUP_E], f16, tag="eaT")
                nc.sync.dma_start(eaT[:], ea_d[:, g * GROUP_E:(g + 1) * GROUP_E])
                oh_sb = sb.tile([16, GROUP_E], f16, tag="oh")
                nc.sync.dma_start(oh_sb[:], oh_d[:, g * GROUP_E:(g + 1) * GROUP_E])
                sm_sb = sb.tile([1, GROUP_E], f16, tag="sm")
                nc.sync.dma_start(sm_sb[:], sm_d[:, g * GROUP_E:(g + 1) * GROUP_E])

                outT = op.tile([128, GROUP_E], f16, tag="outT")
                if skip_compute:
                    nc.vector.tensor_add(outT[:, :GROUP_E], dstT[:, 0, :],
                                         eaT[:])
                else:
                    for t in range(GROUP_TILES):
                        sl = slice(t * TILE_E, (t + 1) * TILE_E)
                        # S[p, e] = (srcoff[e] & 127 == p)
                        bc_ps = ps.tile([128, TILE_E], f32, tag="bc")
                        nc.tensor.matmul(bc_ps[:], ones1[:], sm_sb[:, sl],
                                         start=True, stop=True)
                        S_sb = hp.tile([128, TILE_E], f16, tag="S")
                        nc.vector.tensor_scalar(
                            S_sb[:], bc_ps[:], iota_sb[:], None,
                            mybir.AluOpType.is_equal)
                        # srcT = xs_band^T @ S  (= x16[src[e]] columns)
                        x_ps = ps.tile([128, TILE_E], f32, tag="x")
                        nc.tensor.matmul(x_ps[:], xs_sb[:, t, :], S_sb[:],
                                         start=True, stop=True)
                        srcT = hp.tile([128, TILE_E], f16, tag="srcT")
                        nc.vector.tensor_copy(srcT[:], x_ps[:])

                        h_ps = ps.tile([128, TILE_E], f32, tag="h")
                        nc.tensor.matmul(h_ps[:], w1f[:, 0, :], srcT[:],
                                         start=True, stop=False)
                        nc.tensor.matmul(h_ps[:], wu_sb[:], oh_sb[:, sl],
                                         start=False, stop=False)
                        nc.tensor.matmul(h_ps[:], w1f[:, 1, :], dstT[:, 0, sl],
                                         start=False, stop=False)
                        nc.tensor.matmul(h_ps[:], w1f[:, 2, :], eaT[:, sl],
                                         start=False, stop=True)

                        hT = hp.tile([128, TILE_E], f16, tag="hT")
                        nc.scalar.activation(hT[:], h_ps[:],
                                             mybir.ActivationFunctionType.Relu,
                                             bias=b1c[:])

                        o_ps = ps.tile([128, TILE_E], f32, tag="o")
                        nc.tensor.matmul(o_ps[:], w2f[:], hT[:],
                                         start=True, stop=True)
                        nc.scalar.activation(outT[:, sl], o_ps[:],
                                             mybir.ActivationFunctionType.Relu,
                                             bias=b2c[:])

                if skip_stores:
                    nc.sync.dma_start(out_d[:, g * GROUP_E:g * GROUP_E + 128],
                                      outT[:, :128])
                else:
                    nc.sync.dma_start(out_d[:, g * GROUP_E:(g + 1) * GROUP_E],
                                      outT[:])

        if reps == 1:
            emit_main()
        else:
            with tc.For_i(0, reps, 1):
                emit_main()

    nc.compile()
    return nc


def _prep_inputs(x, edge_attr, u, W1, b1, W2, b2, edge_index, batch):
    src = np.asarray(edge_index[0]).astype(np.int64)
    dst = np.asarray(edge_index[1]).astype(np.int64)
    batch = np.asarray(batch).astype(np.int64)

    x16 = np.asarray(x, np.float16)
    tbs0 = np.zeros((SRC_ROWS, D), np.float16)
    tbs0[:SPLIT] = x16[:SPLIT]
    tbs1 = np.zeros((SRC_ROWS, D), np.float16)
    tbs1[:N_NODES - SPLIT] = x16[SPLIT:]
    tbd0 = np.zeros((TBL_PAD, D), np.float16)
    tbd0[:SPLIT] = x16[:SPLIT]
    tbd1 = np.zeros((TBL_PAD, D), np.float16)
    tbd1[:N_NODES - SPLIT] = x16[SPLIT:]

    srcoff = np.where(src >= SPLIT, src - SPLIT, src)
    dstoff = np.where(dst >= SPLIT, dst - SPLIT, dst)
    bucket = (src >= SPLIT) * 2 + (dst >= SPLIT)
    band = srcoff >> 7                          # 0..195

    key = bucket * 256 + band
    order = np.argsort(key, kind="stable")
    cnt = np.bincount(key, minlength=4 * 256).reshape(4, 256)

    perm = np.full((N_CORES, SLOTS), -1, np.int64)
    overflow = []
    pos = 0
    for b in range(4):
        for bd in range(256):
            n = cnt[b, bd]
            if n == 0:
                continue
            ids = order[pos:pos + n]
            pos += n
            n0 = min((n + 1) // 2, TILE_E)
            n1 = min(n - n0, TILE_E)
            if n0 + n1 < n:
                overflow.append(ids[n0 + n1:])
            g, t = bd // GROUP_TILES, bd % GROUP_TILES
            base = g * GROUP_E + t * TILE_E
            perm[2 * b, base:base + n0] = ids[:n0]
            perm[2 * b + 1, base:base + n1] = ids[n0:n0 + n1]
    overflow = np.concatenate(overflow) if overflow else np.zeros(0, np.int64)

    ea16 = np.asarray(edge_attr, np.float16)
    W1f = np.asarray(W1, np.float32)
    W2f = np.asarray(W2, np.float32)
    b1f = np.asarray(b1, np.float32)
    b2f = np.asarray(b2, np.float32)
    uf = np.asarray(u, np.float32)
    iota = np.arange(128, dtype=np.float32).reshape(128, 1)

    in_maps = []
    for c in range(N_CORES):
        bs, bd_half = (c // 2) >> 1, (c // 2) & 1
        pc = perm[c]
        valid = pc >= 0
        pv = np.where(valid, pc, 0)

        didx = np.where(valid, dstoff[pv], 0).astype(np.int16)
        sm = np.where(valid, srcoff[pv] & 127, 300).astype(np.float16)
        eac = ea16[pv]
        eac[~valid] = 0
        bsrc = batch[src[pv]]
        ohc = (np.arange(16)[:, None] == bsrc[None, :]).astype(np.float16)

        im = {
            "tbs": tbs1 if bs else tbs0,
            "tbd": tbd1 if bd_half else tbd0,
            "ea": np.ascontiguousarray(eac.T),
            "oh": np.ascontiguousarray(ohc),
            "sm": np.ascontiguousarray(sm.reshape(1, SLOTS)),
            "idx": np.ascontiguousarray(
                _wrap_idx(didx.reshape(N_GROUPS, GROUP_E))),
            "iota": iota,
            "w1": W1f, "w2": W2f, "b1": b1f, "b2": b2f, "u": uf,
        }
        in_maps.append(im)
    return in_maps, perm.reshape(-1), overflow


def _cpu_edges(ids, x, edge_attr, u, W1, b1, W2, b2, src, dst, batch):
    agg = np.concatenate([x[src[ids]], x[dst[ids]], edge_attr[ids],
                          u[batch[src[ids]]]], axis=1).astype(np.float32)
    h = np.maximum(agg @ W1 + b1, 0)
    return np.maximum(h @ W2 + b2, 0)


def kernel(x, edge_attr, u, W1, b1, W2, b2, edge_index, batch):
    if "nc" not in _CACHE:
        _CACHE["nc"] = _build_program()
    nc = _CACHE["nc"]
    in_maps, perm, overflow = _prep_inputs(x, edge_attr, u, W1, b1, W2, b2,
                                           edge_index, batch)
    res = run_bass_kernel_spmd(nc, in_maps, list(range(N_CORES)))
    outT = np.concatenate([r["out"] for r in res.results], axis=1)

    out = np.zeros((N_EDGES, D), np.float32)
    valid = perm >= 0
    out[perm[valid]] = outT.T[valid].astype(np.float32)
    if len(overflow):
        src = np.asarray(edge_index[0]).astype(np.int64)
        dst = np.asarray(edge_index[1]).astype(np.int64)
        out[overflow] = _cpu_edges(
            overflow, np.asarray(x, np.float32),
            np.asarray(edge_attr, np.float32),
            np.asarray(u, np.float32), np.asarray(W1, np.float32),
            np.asarray(b1, np.float32), np.asarray(W2, np.float32),
            np.asarray(b2, np.float32), src, dst,
            np.asarray(batch).astype(np.int64))
    return out
